# revision 60
# baseline (speedup 1.0000x reference)
"""Cross-attention kernel for 8 Trainium2 NeuronCores.

Sharding: 16 heads -> 2 heads per core (Megatron column-parallel QKV, row-
parallel out-projection). Each core computes its two heads' attention for both
batch elements and a partial (dmodel x tokens) output; the host sums the 8
partials (shipped bf16) and adds the bias.

Dataflow is feature-major ("transposed") end to end:
  xT/ctxT [1024, 4096] -> qT/kT/vT [128(hd), 4096] -> scoresT [j, i]
so nothing needs transposing except V (done on the PE), and the softmax
denominator falls out of the attn@V matmul as a 65th output row (ones column
appended to V). Matmul operands are bf16 (fp32 PSUM accumulate). The two
heads' K=64 score matmuls run as a row-tiled concurrent pair (auto
tile_position from the base-64 partition slice), as do the per-head K=64
out-projection matmuls in the tail.

The emission order is a manual software pipeline: the PE stream must never
wait on the (serial, in-order) ScalarE exp stream, so score matmuls for group
g+1 are emitted before attn@V of group g (double-buffered score PSUM), and
out-projection / late projection-chunk work is woven into the attention group
loop as PE filler, paced by PE-cost against a backward-smoothed deadline
curve so no group carries much more than ~1.1us of filler burst (an
unsmoothed burst starves the exp stream and shows up as a 2-4us hole).
Chunk inputs stream on both genuinely-parallel DMA paths (SP HWDGE ring and
SWDGE; the two HWDGE rings serialize against each other), split into
contraction halves so projection halves start as soon as half the data
lands, and a throwaway matmul burst holds the PE HAM clock-gate open across
the initial DMA wait. The softmax reciprocal is re-laid out to 128
partitions via a DRAM bounce (a [1, 1024] single-partition reciprocal costs
6.5us on DVE; [128, 8] costs ~50ns).
"""

import numpy as np

B, N, D, H, DH = 2, 2048, 1024, 16, 64
SCALE = DH ** -0.5
NTOK = B * N            # 4096
HDC = 2 * DH            # 128 head-dims per core (2 heads)
NCORES = 8

TOKCHUNK = 512          # projection chunk (8 chunks; 0-3 up front, 4-7 woven in)
ICHUNK = 512            # query chunk in attention (4 per batch)
NJT = N // 128          # 16 j-tiles per batch
KT = D // 128           # 8 contraction tiles for projections

_PROGRAM = None


def _build_program():
    from contextlib import ExitStack
    import concourse.mybir as mybir
    import concourse.tile as tile
    from concourse import bacc
    from concourse.masks import make_identity

    F32 = mybir.dt.float32
    F32R = mybir.dt.float32r
    F16 = mybir.dt.float16
    BF16 = mybir.dt.bfloat16
    AF = mybir.ActivationFunctionType

    nc = bacc.Bacc(None, target_bir_lowering=False)

    NCH = NTOK // TOKCHUNK
    xt_e = nc.declare_dram_parameter("xt", [NCH, 128, KT, TOKCHUNK], BF16,
                                     isOutput=False)
    ct_e = nc.declare_dram_parameter("ct", [NCH, 128, KT, TOKCHUNK], BF16,
                                     isOutput=False)
    wq_e = nc.declare_dram_parameter("wq", [D, HDC], BF16, isOutput=False)
    wk_e = nc.declare_dram_parameter("wk", [D, HDC], BF16, isOutput=False)
    wv_e = nc.declare_dram_parameter("wv", [D, HDC], BF16, isOutput=False)
    wo_e = nc.declare_dram_parameter("wo", [HDC, D], BF16, isOutput=False)
    out_e = nc.declare_dram_parameter("out", [D, NTOK], BF16, isOutput=True)
    # last i-chunk ships unnormalized per-head products + rowsums; the host
    # divides and sums (kills the serial norm-chain + outproj tail latency)
    tail_e = nc.declare_dram_parameter("tail", [2 * D, ICHUNK], BF16,
                                       isOutput=True)
    tailr_e = nc.declare_dram_parameter("tailr", [2, ICHUNK], F32,
                                        isOutput=True)

    wq_v = wq_e[:].rearrange("(t p) m -> p t m", p=128)     # [128, 8, 128]
    wk_v = wk_e[:].rearrange("(t p) m -> p t m", p=128)
    wv_v = wv_e[:].rearrange("(t p) m -> p t m", p=128)
    out_v = out_e[:].rearrange("(t p) n -> p t n", p=128)   # [128, 8, 4096]

    with tile.TileContext(nc) as tc, ExitStack() as ctx:
        const = ctx.enter_context(tc.tile_pool(name="const", bufs=1))
        wpool = ctx.enter_context(tc.tile_pool(name="wpool", bufs=1))
        xsp = ctx.enter_context(tc.tile_pool(name="xsp", bufs=4))
        csp = ctx.enter_context(tc.tile_pool(name="csp", bufs=4))
        qkp = ctx.enter_context(tc.tile_pool(name="qkp", bufs=1))
        vtp = ctx.enter_context(tc.tile_pool(name="vtp", bufs=3))
        vsb = ctx.enter_context(tc.tile_pool(name="vsb", bufs=1))
        exp = ctx.enter_context(tc.tile_pool(name="exp", bufs=4))
        nrm = ctx.enter_context(tc.tile_pool(name="nrm", bufs=2))
        obp = ctx.enter_context(tc.tile_pool(name="obp", bufs=4))
        drp = ctx.enter_context(tc.tile_pool(name="drp", bufs=2, space="DRAM"))
        ps_s = ctx.enter_context(tc.tile_pool(name="ps_s", bufs=2, space="PSUM"))
        ps_a = ctx.enter_context(tc.tile_pool(name="ps_a", bufs=1, space="PSUM"))
        ps_m = ctx.enter_context(tc.tile_pool(name="ps_m", bufs=2, space="PSUM"))

        # --- constants ---
        ones32 = const.tile([128, 128], F32, tag="ones32", name="ones32")
        nc.gpsimd.memset(ones32[:], 1.0)
        ident32 = const.tile([128, 128], F32, tag="ident32", name="ident32")
        make_identity(nc, ident32)
        ident = const.tile([128, 128], BF16, tag="ident", name="ident")
        nc.vector.tensor_copy(ident[:], ident32[:])
        wrm = const.tile([128, 512], BF16, tag="wrm", name="wrm")
        nc.vector.memset(wrm[:], 0.0)

        # --- weights + chunk-0 inputs across THREE parallel DMA paths:
        # sync (SP HWDGE ring) carries wk + x chunks, scalar (ACT HWDGE
        # ring, idle until the first exp) carries the critical ctx chunk 0,
        # gpsimd (SWDGE) carries wv/wo + later ctx chunks. Single-queue
        # serialization of ~2.75MB was the old 30us+ prologue.
        wq_sb = wpool.tile([128, KT, HDC], BF16, tag="wq_sb", name="wq_sb")
        wk_sb = wpool.tile([128, KT, HDC], BF16, tag="wk_sb", name="wk_sb")
        wv_sb = wpool.tile([128, KT, HDC], BF16, tag="wv_sb", name="wv_sb")
        wo_sb = wpool.tile([128, D], BF16, tag="wo_sb", name="wo_sb")
        # The two HWDGE rings (SP and ACT) serialize against each other in
        # practice, so the usable parallel paths are the SP HWDGE ring and
        # SWDGE. The k-side criticals (wk + ctx chunk 0) stream on HWDGE
        # while the q-side (wq + x chunk 0) streams on SWDGE concurrently;
        # contraction halves let each projection half start as soon as its
        # half of the data lands.
        KH = KT // 2
        nc.sync.dma_start(wk_sb[:], wk_v)
        cs0 = csp.tile([128, KT, TOKCHUNK], BF16, tag="cs", name="cs0")
        nc.sync.dma_start(cs0[:, 0:KH], ct_e[0][:, 0:KH])
        nc.sync.dma_start(cs0[:, KH:], ct_e[0][:, KH:])
        xs0 = xsp.tile([128, KT, TOKCHUNK], BF16, tag="xs", name="xs0")
        nc.sync.dma_start(xs0[:, 0:KH], xt_e[0][:, 0:KH])
        nc.sync.dma_start(xs0[:, KH:], xt_e[0][:, KH:])
        nc.gpsimd.dma_start(wq_sb[:], wq_v)
        nc.gpsimd.dma_start(wv_sb[:], wv_v)
        # wo isn't read until the first out-projection; it trails the
        # chunk-0 criticals on the SP ring
        nc.sync.dma_start(wo_sb[:], wo_e[:])

        # a short throwaway matmul burst keeps the PE HAM activity window
        # non-idle while the chunk-0 DMAs land, so the clock-gate releases
        # during (not after) the first projections; sized to finish before
        # cs0 arrives so it never delays real work
        warm_ps = ps_m.tile([128, 512], F32, tag="misc", name="warmup")
        for _ in range(10):
            nc.tensor.matmul(warm_ps[:], wrm[:, 0:128], wrm[:],
                             start=True, stop=True)

        # --- persistent activations ---
        qT_sb = qkp.tile([128, NTOK], BF16, tag="qT_sb", name="qT_sb")
        kT_sb = qkp.tile([128, NTOK], BF16, tag="kT_sb", name="kT_sb")
        v_sb = {}
        for b in range(B):
            for h in range(2):
                t = vsb.tile([128, NJT * 65], BF16, tag=f"v{b}{h}", name=f"v{b}{h}")
                v_sb[(b, h)] = t
                ones_col = t.rearrange("p (j c) -> p j c", c=65)[:, :, 64]
                nc.vector.tensor_copy(ones_col, ones32[:, 0:NJT])

        # ---------- projection chunk emission, split into filler-sized pieces
        def chunk_pieces(c):
            """Return a list of closures emitting chunk c's projections +
            V-transposes in ~0.5us PE pieces. Closure list order matters."""
            sl = slice(c * TOKCHUNK, (c + 1) * TOKCHUNK)
            state = {"xs": xs0, "cs": cs0} if c == 0 else {}

            def dma_in():
                # context on SWDGE, x on the SP HWDGE ring: the two paths
                # genuinely overlap
                cs = csp.tile([128, KT, TOKCHUNK], BF16, tag="cs", name=f"cs{c}")
                nc.gpsimd.dma_start(cs[:], ct_e[c])
                xs = xsp.tile([128, KT, TOKCHUNK], BF16, tag="xs", name=f"xs{c}")
                nc.sync.dma_start(xs[:], xt_e[c])
                state["xs"], state["cs"] = xs, cs

            def proj(kind, half):
                """Half of a projection: 4 accumulating N=512 matmuls (full
                PE efficiency; N=256 was LDWEIGHTS-issue-bound). Emitted as
                two pieces so a filler burst never exceeds ~1us of PE time
                between two score groups; the second half finishes the PSUM
                accumulation group and evacuates. The two halves must stay
                adjacent in the filler queue (same pool buffer)."""
                w, src, dst = {
                    "q": (wq_sb, "xs", qT_sb),
                    "k": (wk_sb, "cs", kT_sb),
                    "v": (wv_sb, "cs", None),
                }[kind]
                if half == 0:
                    p = ps_m.tile([128, TOKCHUNK], F32, tag="misc",
                                  name=f"p{kind}{c}")
                    state[f"ps{kind}"] = p
                    for t in range(KT // 2):
                        nc.tensor.matmul(p[:], w[:, t, :], state[src][:, t, :],
                                         start=(t == 0), stop=False)
                    return
                p = state[f"ps{kind}"]
                for t in range(KT // 2, KT):
                    nc.tensor.matmul(p[:], w[:, t, :], state[src][:, t, :],
                                     start=False, stop=(t == KT - 1))
                if kind == "v":
                    if "vt" not in state:
                        state["vt"] = vtp.tile([128, TOKCHUNK], BF16, tag="vt",
                                               name=f"vt{c}")
                    nc.vector.tensor_copy(state["vt"][:], p[:])
                else:
                    gsl = slice(c * TOKCHUNK, (c + 1) * TOKCHUNK)
                    nc.vector.tensor_copy(dst[:, gsl], p[:])

            def vtrans(jj):
                b = c // 4
                jt = (c % 4) * 4 + jj
                pt = ps_m.tile([128, 128], BF16, tag="misc", name=f"pt{c}_{jj}")
                nc.tensor.transpose(
                    pt[:], state["vt"][:, jj * 128:(jj + 1) * 128], ident[:])
                for h in range(2):
                    nc.vector.tensor_copy(
                        v_sb[(b, h)][:, 65 * jt: 65 * jt + 64],
                        pt[:, 64 * h: 64 * h + 64])

            # (deadline_group, closure): deadline = global group index
            # (16 per iteration) by which the piece must be EMITTED
            # (program order defines dependencies).
            # batch b0 chunk c: k/v/trans feed j-groups 4c.. of iter (0,0);
            # q feeds iteration (0,c). b1 chunk c: k/v/trans feed iter (1,0)
            # = group 64; q feeds iteration (1,c-4) = group 16c.
            # Deadlines are backstops only; the cost-weighted pacer below
            # pulls pieces ahead into whatever PE slack exists, so deadlines
            # carry a ~6-group margin before the group that consumes them.
            if c < 4:
                kvdl = 4 * c
                qdl = 16 * c - 8 if c >= 1 else 0
            else:
                kvdl = 58 + 4 * (c - 4)
                qdl = 16 * c - 8
            # tuple: (deadline, closure, opens_psum_half, PE_cost_us)
            pieces = []
            if c > 0:
                ddl = max(0, kvdl - 6)
                pieces.append((ddl, dma_in, False, 0.05 if ddl else 0.0))
            for kind in ("k", "q", "v"):
                dl = qdl if kind == "q" else kvdl
                cost = 1.0 if dl > 0 else 0.0   # prologue pieces uncharged
                pieces.append((dl, lambda k=kind: proj(k, 0), True, cost))
                pieces.append((dl, lambda k=kind: proj(k, 1), False, cost))
            for jj in range(TOKCHUNK // 128):
                pieces.append((kvdl, lambda j=jj: vtrans(j), False,
                               0.3 if kvdl else 0.0))
            return pieces

        # ---------- out-projection pieces for one finished iteration
        def outproj_pieces(b, i, on, last):
            isl = slice(b * N + i * ICHUNK, b * N + (i + 1) * ICHUNK)

            def one(d8):
                po = ps_m.tile([128, ICHUNK], F32, tag="misc",
                               name=f"po{b}_{i}_{d8}")
                nc.tensor.matmul(po[:], wo_sb[:, d8 * 128:(d8 + 1) * 128], on[:],
                                 start=True, stop=True)
                ob = obp.tile([128, ICHUNK], BF16, tag="ob", name=f"ob{b}_{i}_{d8}")
                with nc.allow_low_precision(reason="out partial bf16"):
                    nc.vector.tensor_copy(ob[:], po[:])
                # alternate both DMA rings so neither queue backs up behind
                # the bulk output writes (~8.5MB total at bf16)
                eng = nc.sync if d8 % 2 == 0 else nc.gpsimd
                eng.dma_start(out_v[:, d8, isl], ob[:])

            return [lambda d=d8: one(d) for d8 in range(8)]

        # ---------- emission schedule ----------
        # chunk 0 is the prologue (iteration (0,0) group 0 needs it whole);
        # everything else is deadline-paced into the attention group loop.
        cq = []               # chunk pieces: (deadline_group, closure)
        oq = []               # out-projection pieces (always safe to defer)
        for c in range(8):
            cq.extend(chunk_pieces(c))
        # stable-sort by deadline so the head of the queue is always the
        # most urgent piece (intra-chunk emission order is preserved)
        cq.sort(key=lambda t: t[0])

        iters = [(b, i) for b in range(B) for i in range(N // ICHUNK)]
        pend_scores = None    # emitted-but-unconsumed scores group closure
        # Adaptive pacing: the filler demand is far denser during b0 (b1's
        # k/v work can be pre-built) than b1, so a flat rate guarantees
        # deadline bursts. Target the cumulative PE-cost curve implied by
        # the deadlines with a 4-group lookahead, floored by an even spread.
        total_cost = sum(p[3] for p in cq)
        req = [0.0] * 129
        for dl, _, _, cost in cq:
            req[min(dl, 128)] += cost
        for g in range(1, 129):
            req[g] += req[g - 1]
        # backward pass: emit as late as deadlines allow, but never demand
        # more than ~1.1us of filler PE in one group (except where the
        # deadline density itself is higher, e.g. the first-iteration ramp)
        tgt = [0.0] * 129
        tgt[128] = total_cost
        for g in range(127, -1, -1):
            tgt[g] = max(req[min(g + 4, 128)], tgt[g + 1] - 1.1)
        spent = [0.0]
        half_open = [False]   # a proj half1 was popped; half2 must follow

        def pop_cq():
            _, fn, h1, cost = cq.pop(0)
            fn()
            half_open[0] = h1
            spent[0] += cost

        for it, (b, i) in enumerate(iters):
            isl = slice(b * N + i * ICHUNK, b * N + (i + 1) * ICHUNK)
            acc = ps_a.tile([128, 2 * ICHUNK], F32, tag="acc", name=f"acc{b}_{i}")

            # one group = one j-tile, both heads (row-packed score pair)
            def scores(b, i, j, isl):
                ss = ps_s.tile([128, 2 * 512], F32, tag="ss", name=f"ss{b}_{i}_{j}")
                jsl = slice(b * N + j * 128, b * N + (j + 1) * 128)
                for h in range(2):
                    hs = slice(64 * h, 64 * h + 64)
                    nc.tensor.matmul(ss[:, 512 * h: 512 * (h + 1)],
                                     kT_sb[hs, jsl], qT_sb[hs, isl],
                                     start=True, stop=True)
                ex = exp.tile([128, 2 * 512], BF16, tag="ex", name=f"ex{b}_{i}_{j}")
                nc.scalar.activation(ex[:], ss[:], AF.Exp)
                return ex

            def attnv(b, i, j, ex, acc):
                for h in range(2):
                    nc.tensor.matmul(
                        acc[0:65, ICHUNK * h: ICHUNK * (h + 1)],
                        v_sb[(b, h)][:, 65 * j: 65 * j + 65],
                        ex[:, 512 * h: 512 * (h + 1)],
                        start=(j == 0), stop=(j == NJT - 1))

            for g in range(NJT):
                gg_global = it * NJT + g
                # correctness: pieces this group's scores/attnv read must be
                # emitted first (program order = dependency order)
                while cq and (cq[0][0] <= gg_global or half_open[0]):
                    pop_cq()
                ex = scores(b, i, g, isl)
                # consume previous group (scores g+1 emitted before attnv g)
                if pend_scores is not None:
                    pend_scores()
                pend_scores = (lambda bb=b, ii=i, gg=g, e=ex, a=acc:
                               attnv(bb, ii, gg, e, a))
                # fillers: outproj pieces on odd groups, chunk pieces paced
                # toward the deadline-density target; an open proj half
                # always closes before anything else can touch ps_m
                if half_open[0] and cq:
                    pop_cq()
                elif g % 2 == 1 and oq:
                    oq.pop(0)()
                while cq and spent[0] < tgt[min(gg_global, 128)]:
                    pop_cq()
                if half_open[0] and cq:
                    pop_cq()
            pend_scores()
            pend_scores = None

            if it == len(iters) - 1:
                # ---- tail: ship unnormalized per-head products + rowsums;
                # host normalizes. No serial norm chain at the very end.
                # ScalarE is idle after the last exp, so it takes half the
                # PSUM evacuations (ACT Copy) to pipeline with the DVE.
                # A few dependency-free matmuls keep the PE HAM window busy
                # across the last exp/evac wait so the 16 tail matmuls run
                # at full clock instead of re-throttled half clock. Fresh
                # pool tiles keep the ps_m rotation's WAW tracking sound.
                for k in range(6):
                    wp = ps_m.tile([128, 512], F32, tag="misc",
                                   name=f"tailwarm{k}")
                    nc.tensor.matmul(wp[:], wrm[:, 0:128], wrm[:],
                                     start=True, stop=True)
                on_un = nrm.tile([128, ICHUNK], BF16, tag="on_un", name="on_un")
                nc.vector.tensor_copy(on_un[0:64, :], acc[0:64, 0:ICHUNK])
                nc.scalar.activation(on_un[64:128, :], acc[0:64, ICHUNK:],
                                     AF.Copy)
                # multi-partition evac, then DMA picks out the rowsum row
                ta_sb = nrm.tile([128, 2 * ICHUNK], F32, tag="ta_sb", name="ta_sb")
                nc.scalar.activation(ta_sb[0:65, :], acc[0:65, :], AF.Copy)
                nc.scalar.dma_start(
                    tailr_e[:].rearrange("a b -> (a b)"), ta_sb[64:65, :])
                tail_v = tail_e[:].rearrange("(h t p) n -> h p t n",
                                             h=2, p=128)
                # per d8-block the two heads' K=64 matmuls form a row-tiled
                # concurrent pair (h0 rows 0-63, h1 rows 64-127, separate
                # PSUM buffers); evacuations split across DVE and the idle
                # ScalarE, DMAs across both rings
                for d8 in range(8):
                    po = [None, None]
                    for h in range(2):
                        po[h] = ps_m.tile([128, ICHUNK], F32, tag="misc",
                                          name=f"tpo{h}_{d8}")
                        nc.tensor.matmul(
                            po[h][:], wo_sb[64 * h: 64 * h + 64,
                                            d8 * 128:(d8 + 1) * 128],
                            on_un[64 * h: 64 * h + 64, :],
                            start=True, stop=True)
                    for h in range(2):
                        ob = obp.tile([128, ICHUNK], BF16, tag="obt",
                                      name=f"tob{h}_{d8}")
                        if h == 0:
                            nc.vector.tensor_copy(ob[:], po[h][:])
                        else:
                            nc.scalar.activation(ob[:], po[h][:], AF.Copy)
                        eng = nc.sync if h == 0 else nc.gpsimd
                        eng.dma_start(tail_v[h, :, d8, :], ob[:])
                continue

            # ---- normalization (latency-tolerant; consumed by next iter's
            # outproj fillers) ----
            accs = nrm.tile([128, 2 * ICHUNK], F32R, tag="accs", name=f"accs{b}_{i}")
            with nc.allow_low_precision(reason="attn out + softmax denom fp32r"):
                nc.vector.tensor_copy(accs[0:65, 0:ICHUNK], acc[0:65, 0:ICHUNK])
                nc.vector.tensor_copy(accs[0:65, ICHUNK:], acc[0:65, ICHUNK:])
            # rowsum [1, 1024]@p64 -> DRAM -> [128, 8] -> recip -> DRAM ->
            # broadcast-DMA to all 128 partitions (no PE, no PSUM involved)
            rs_d = drp.tile([2 * ICHUNK], F32R, tag="rs_d", name=f"rs_d{b}_{i}")
            nc.sync.dma_start(rs_d[:], accs[64:65, :])
            rs128 = nrm.tile([128, 8], F32R, tag="rs128", name=f"rs128{b}_{i}")
            nc.sync.dma_start(rs128[:], rs_d[:].rearrange("(p a) -> p a", p=128))
            rr128 = nrm.tile([128, 8], F32R, tag="rr128", name=f"rr128{b}_{i}")
            with nc.allow_low_precision(reason="softmax denom recip fp32r"):
                nc.vector.reciprocal(rr128[:], rs128[:])
            rr_d = drp.tile([2 * ICHUNK], F32R, tag="rr_d", name=f"rr_d{b}_{i}")
            nc.sync.dma_start(rr_d[:].rearrange("(p a) -> p a", p=128), rr128[:])
            bcs = nrm.tile([128, 2 * ICHUNK], F32R, tag="bcs", name=f"bcs{b}_{i}")
            nc.sync.dma_start(
                bcs[:], rr_d[:].unsqueeze(0).broadcast_to([128, 2 * ICHUNK]))

            on = nrm.tile([128, ICHUNK], BF16, tag="on", name=f"on{b}_{i}", bufs=5)
            with nc.allow_low_precision(reason="attn out normalize bf16"):
                for h in range(2):
                    nc.vector.tensor_mul(
                        on[64 * h: 64 * h + 64, :],
                        accs[0:64, ICHUNK * h: ICHUNK * (h + 1)],
                        bcs[0:64, ICHUNK * h: ICHUNK * (h + 1)])

            # pieces for iteration `it` pop during iteration it+1, so the
            # ones emitted at it == len-2 are the ones draining at the end
            oq.extend(outproj_pieces(b, i, on, last=(it == len(iters) - 2)))

        while cq:
            pop_cq()
        while oq:
            oq.pop(0)()

    nc.compile()
    return nc


def _get_program():
    global _PROGRAM
    if _PROGRAM is None:
        _PROGRAM = _build_program()
    return _PROGRAM


def _prepare_in_maps(x, context, Wq, Wk, Wv, Wo, bo):
    import ml_dtypes
    bf16 = ml_dtypes.bfloat16

    x = np.asarray(x, dtype=np.float32)
    context = np.asarray(context, dtype=np.float32)
    Wq = np.asarray(Wq, dtype=np.float32)
    Wk = np.asarray(Wk, dtype=np.float32)
    Wv = np.asarray(Wv, dtype=np.float32)
    Wo = np.asarray(Wo, dtype=np.float32)

    NCH = NTOK // TOKCHUNK
    xT = (x.reshape(NTOK, D).T.reshape(KT, 128, NCH, TOKCHUNK)
          .transpose(2, 1, 0, 3))
    xT = np.ascontiguousarray(xT).astype(bf16)
    cT = (context.reshape(NTOK, D).T.reshape(KT, 128, NCH, TOKCHUNK)
          .transpose(2, 1, 0, 3))
    cT = np.ascontiguousarray(cT).astype(bf16)
    Wk_s = Wk * np.float32(SCALE)

    in_maps = []
    for c in range(NCORES):
        cs = slice(c * HDC, (c + 1) * HDC)
        in_maps.append({
            "xt": xT,
            "ct": cT,
            "wq": np.ascontiguousarray(Wq[:, cs]).astype(bf16),
            "wk": np.ascontiguousarray(Wk_s[:, cs]).astype(bf16),
            "wv": np.ascontiguousarray(Wv[:, cs]).astype(bf16),
            "wo": np.ascontiguousarray(Wo[cs, :]).astype(bf16),
        })
    return in_maps


def _gather(results, bo):
    bo = np.asarray(bo, dtype=np.float32)
    acc = results[0]["out"].astype(np.float64)
    for c in range(1, NCORES):
        acc += results[c]["out"].astype(np.float64)
    # last i-chunk arrives unnormalized per head + rowsums; divide and sum
    last = slice(NTOK - ICHUNK, NTOK)
    for c in range(NCORES):
        tail = results[c]["tail"].astype(np.float64)
        tailr = results[c]["tailr"].astype(np.float64)
        tA, tB = tail[0:D], tail[D:2 * D]
        rA, rB = tailr[0], tailr[1]
        acc[:, last] += tA / rA + tB / rB
    out = acc.T.astype(np.float32) + bo
    return out.reshape(B, N, D)


def kernel(x, context, Wq, Wk, Wv, Wo, bo):
    from concourse.bass_utils import run_bass_kernel_spmd

    in_maps = _prepare_in_maps(x, context, Wq, Wk, Wv, Wo, bo)
    nc = _get_program()
    res = run_bass_kernel_spmd(nc, in_maps, list(range(NCORES)))
    return _gather(res.results, bo)

